# revision 1
# baseline (speedup 1.0000x reference)
from contextlib import ExitStack

import numpy as np

import concourse.bass as bass
import concourse.mybir as mybir
import concourse.tile as tile
from concourse import bacc
from concourse.bass_utils import run_bass_kernel_spmd

F32 = mybir.dt.float32
F32R = mybir.dt.float32r
AF = mybir.ActivationFunctionType
ALU = mybir.AluOpType

T = 4096
D = 2048
SEG = 512
NSEG = T // SEG
NH = 8
DK = 128
DV = 128
TC = NSEG * 256
NKK = D // 128
SCALE = 1.0 / np.sqrt(128.0)


def build_core_program():
    nc = bacc.Bacc("TRN2", target_bir_lowering=False)

    x = nc.dram_tensor("x", [TC, D], F32, kind="ExternalInput")
    wq = nc.dram_tensor("wq", [D, D], F32, kind="ExternalInput")
    wk = nc.dram_tensor("wk", [D, D], F32, kind="ExternalInput")
    wv = nc.dram_tensor("wv", [D, D], F32, kind="ExternalInput")
    wo = nc.dram_tensor("wo", [D, D], F32, kind="ExternalInput")
    bg_cols = nc.dram_tensor("bg_cols", [DV, NH], F32, kind="ExternalInput")
    omb_cols = nc.dram_tensor("omb_cols", [DV, NH], F32, kind="ExternalInput")
    ident_in = nc.dram_tensor("ident", [128, 128], F32, kind="ExternalInput")
    ones_in = nc.dram_tensor("ones_in", [128, 128], F32, kind="ExternalInput")
    zeros_in = nc.dram_tensor("zeros_in", [128, NH * DV + NH], F32, kind="ExternalInput")
    out = nc.dram_tensor("out", [TC, D], F32, kind="ExternalOutput")

    with ExitStack() as ctx:
        ctx.enter_context(
            nc.allow_low_precision(reason="float32r tiles feed the PE; fp32 bits")
        )
        tc = ctx.enter_context(tile.TileContext(nc))

        dram = ctx.enter_context(tc.tile_pool(name="dram", bufs=1, space="DRAM"))
        qT_scr = dram.tile([NKK, 128, TC], F32)
        kT_scr = dram.tile([NKK, 128, TC], F32)
        v_scr = dram.tile([TC, D], F32)
        af_scr = dram.tile([NSEG, 128, NKK, 256], F32)

        const = ctx.enter_context(tc.tile_pool(name="const", bufs=1))
        ident = const.tile([128, 128], F32R)
        nc.sync.dma_start(out=ident, in_=ident_in[:, :].bitcast(F32R))
        ones_col = const.tile([128, 1], F32R)
        nc.sync.dma_start(out=ones_col, in_=ones_in[:, 0:1].bitcast(F32R))
        ones_row = const.tile([1, 128], F32R)
        nc.sync.dma_start(out=ones_row, in_=ones_in[0:1, :].bitcast(F32R))

        for which, w_dram in (("q", wq), ("k", wk), ("v", wv)):
            with ExitStack() as p1:
                wpool = p1.enter_context(tc.tile_pool(name=f"w_{which}", bufs=1))
                sb = p1.enter_context(tc.tile_pool(name=f"sb_{which}", bufs=2))
                st_pool = p1.enter_context(tc.tile_pool(name=f"st_{which}", bufs=4))
                ps = p1.enter_context(tc.tile_pool(name=f"ps_{which}", bufs=4, space="PSUM"))
                ps_t = p1.enter_context(tc.tile_pool(name=f"pst_{which}", bufs=4, space="PSUM"))

                wt = wpool.tile([128, NKK, D], F32R, tag="wt")
                nc.sync.dma_start(
                    out=wt, in_=w_dram.rearrange("(kk p) j -> p kk j", p=128).bitcast(F32R)
                )

                for seg in range(NSEG):
                    r0 = seg * 256
                    xT = sb.tile([128, NKK, 256], F32R, tag="xT")
                    for st in range(2):
                        xn = sb.tile([128, D], F32R, tag="xn")
                        nc.sync.dma_start(
                            out=xn, in_=x[r0 + st * 128 : r0 + (st + 1) * 128, :].bitcast(F32R)
                        )
                        for dd in range(NKK):
                            pt = ps_t.tile([128, 128], F32R, tag="tr")
                            nc.tensor.transpose(pt, xn[:, dd * 128 : (dd + 1) * 128], ident)
                            if dd % 2 == 0:
                                nc.vector.tensor_copy(xT[:, dd, st * 128 : (st + 1) * 128], pt)
                            else:
                                nc.scalar.copy(xT[:, dd, st * 128 : (st + 1) * 128], pt)

                    if which in ("q", "k"):
                        scr = qT_scr if which == "q" else kT_scr
                        for jc in range(NKK):
                            pq = ps.tile([128, 256], F32, tag="proj")
                            for kk in range(NKK):
                                nc.tensor.matmul(
                                    pq,
                                    wt[:, kk, jc * 128 : (jc + 1) * 128],
                                    xT[:, kk, :],
                                    start=(kk == 0),
                                    stop=(kk == NKK - 1),
                                )
                            qs = st_pool.tile([128, 256], F32, tag="stage")
                            if jc % 2 == 0:
                                nc.scalar.copy(qs, pq)
                            else:
                                nc.vector.tensor_copy(qs, pq)
                            nc.sync.dma_start(out=scr[jc, :, r0 : r0 + 256], in_=qs)
                    else:
                        for st in range(2):
                            for mc in range(4):
                                pv = ps.tile([128, 512], F32, tag="proj")
                                for kk in range(NKK):
                                    nc.tensor.matmul(
                                        pv,
                                        xT[:, kk, st * 128 : (st + 1) * 128],
                                        wt[:, kk, mc * 512 : (mc + 1) * 512],
                                        start=(kk == 0),
                                        stop=(kk == NKK - 1),
                                    )
                                vs = st_pool.tile([128, 512], F32, tag="stage")
                                if (st + mc) % 2 == 0:
                                    nc.scalar.copy(vs, pv)
                                else:
                                    nc.vector.tensor_copy(vs, pv)
                                nc.sync.dma_start(
                                    out=v_scr[
                                        r0 + st * 128 : r0 + (st + 1) * 128,
                                        mc * 512 : (mc + 1) * 512,
                                    ],
                                    in_=vs,
                                )

        with ExitStack() as p2:
            res = p2.enter_context(tc.tile_pool(name="res", bufs=1))
            qk = p2.enter_context(tc.tile_pool(name="qk", bufs=3))
            work = p2.enter_context(tc.tile_pool(name="work", bufs=2))
            af_pool = p2.enter_context(tc.tile_pool(name="af", bufs=2))
            tiny = p2.enter_context(tc.tile_pool(name="tiny", bufs=2))
            psb = p2.enter_context(tc.tile_pool(name="psb", bufs=5, space="PSUM"))
            psr = p2.enter_context(tc.tile_pool(name="psr", bufs=3, space="PSUM"))

            bgc = res.tile([128, NH], F32)
            nc.sync.dma_start(out=bgc, in_=bg_cols[:, :])
            ombc = res.tile([128, NH], F32)
            nc.sync.dma_start(out=ombc, in_=omb_cols[:, :])

            mem = res.tile([128, NH, DV], F32R)
            nc.sync.dma_start(
                out=mem,
                in_=zeros_in[:, 0 : NH * DV].rearrange("p (h v) -> p h v", h=NH).bitcast(F32R),
            )
            cum = res.tile([128, NH], F32R)
            nc.sync.dma_start(out=cum, in_=zeros_in[:, NH * DV :].bitcast(F32R))

            for seg in range(NSEG):
                r0 = seg * 256
                af = af_pool.tile([128, NKK, 256], F32R, tag="af")
                for h in range(NH):
                    tr0 = r0 + h * 32
                    qT = qk.tile([128, SEG], F32R, tag="qT")
                    nc.sync.dma_start(
                        out=qT.rearrange("d (r t) -> d r t", r=16),
                        in_=qT_scr[:, :, tr0 : tr0 + 32]
                        .rearrange("r d t -> d r t")
                        .bitcast(F32R),
                    )
                    kT = qk.tile([128, SEG], F32R, tag="kT")
                    nc.sync.dma_start(
                        out=kT.rearrange("d (r t) -> d r t", r=16),
                        in_=kT_scr[:, :, tr0 : tr0 + 32]
                        .rearrange("r d t -> d r t")
                        .bitcast(F32R),
                    )
                    vv = qk.tile([128, 4, DV], F32R, tag="vv")
                    v_base = v_scr[tr0 : tr0 + 32, :].rearrange(
                        "t (c rr d) -> rr t c d", rr=4, d=128
                    )
                    for rr in range(4):
                        nc.sync.dma_start(
                            out=vv[rr * 32 : (rr + 1) * 32],
                            in_=v_base[rr].bitcast(F32R),
                        )

                    sqT = work.tile([128, SEG], F32R, tag="sqT")
                    skT = work.tile([128, SEG], F32R, tag="skT")
                    for src, dst in ((qT, sqT), (kT, skT)):
                        tmin = work.tile([128, SEG], F32, tag="tmin")
                        nc.vector.tensor_scalar_min(tmin, src, 0.0)
                        texp = work.tile([128, SEG], F32, tag="texp")
                        nc.scalar.activation(out=texp, in_=tmin, func=AF.Exp)
                        nc.vector.scalar_tensor_tensor(
                            out=dst, in0=src, scalar=0.0, in1=texp,
                            op0=ALU.max, op1=ALU.add,
                        )

                    eT = work.tile([128, 4, SEG], F32R, tag="eT")
                    for c in range(4):
                        pscT = psb.tile([128, SEG], F32, tag="ps")
                        nc.tensor.matmul(
                            pscT, kT[:, c * 128 : (c + 1) * 128], qT, start=True, stop=True
                        )
                        nc.scalar.activation(out=eT[:, c, :], in_=pscT, func=AF.Exp, scale=SCALE)

                    den = psr.tile([1, SEG], F32, tag="row")
                    for c in range(4):
                        nc.tensor.matmul(
                            den, ones_col, eT[:, c, :], start=(c == 0), stop=(c == 3)
                        )
                    r_dot = tiny.tile([1, SEG], F32R, tag="r_dot")
                    nc.vector.reciprocal(r_dot, den)

                    rsq = psr.tile([1, SEG], F32, tag="row")
                    nc.tensor.matmul(rsq, ones_col, sqT, start=True, stop=True)
                    r_mem = tiny.tile([1, SEG], F32R, tag="r_mem")
                    nc.vector.reciprocal(r_mem, rsq)

                    rskc = psb.tile([128, 4], F32, tag="ps")
                    for c in range(4):
                        nc.tensor.matmul(
                            rskc[:, c : c + 1],
                            skT[:, c * 128 : (c + 1) * 128].bitcast(F32),
                            ones_col.bitcast(F32),
                            start=True,
                            stop=True,
                        )
                    rsk = tiny.tile([128, 4], F32, tag="rsk")
                    nc.vector.reciprocal(rsk, rskc)

                    cumT = psr.tile([1, 128], F32R, tag="row")
                    nc.tensor.transpose(cumT, cum[:, h : h + 1], ident)
                    z1row = tiny.tile([1, 128], F32, tag="z1row")
                    nc.scalar.add(z1row, cumT, 1.0)
                    r1row = tiny.tile([1, 128], F32R, tag="r1row")
                    nc.vector.reciprocal(r1row, z1row)
                    z1col = tiny.tile([128, 1], F32, tag="z1col")
                    nc.scalar.add(z1col, cum[:, h : h + 1], 1.0)
                    r1col = tiny.tile([128, 1], F32, tag="r1col")
                    nc.vector.reciprocal(r1col, z1col)
                    bgr1 = tiny.tile([128, 1], F32, tag="bgr1")
                    nc.vector.tensor_mul(bgr1, bgc[:, h : h + 1], r1col)

                    csk = tiny.tile([128, 1], F32, tag="csk")
                    nc.vector.reduce_sum(csk, skT, axis=mybir.AxisListType.X)
                    nc.vector.tensor_add(cum[:, h : h + 1], cum[:, h : h + 1], csk)

                    dotp = psb.tile([128, SEG], F32, tag="ps")
                    for c in range(4):
                        nc.tensor.matmul(
                            dotp, vv[:, c, :], eT[:, c, :], start=(c == 0), stop=(c == 3)
                        )
                    memp = psb.tile([128, SEG], F32, tag="ps")
                    nc.tensor.matmul(memp, mem[:, h, :], sqT, start=True, stop=True)

                    rep_dot_ps = psb.tile([128, SEG], F32, tag="ps")
                    nc.tensor.matmul(rep_dot_ps, ones_row, r_dot, start=True, stop=True)
                    rep_dot = work.tile([128, SEG], F32, tag="rep_dot")
                    nc.scalar.copy(rep_dot, rep_dot_ps)
                    rep_mem_ps = psb.tile([128, SEG], F32, tag="ps")
                    nc.tensor.matmul(rep_mem_ps, ones_row, r_mem, start=True, stop=True)
                    rep_mem = work.tile([128, SEG], F32, tag="rep_mem")
                    nc.scalar.copy(rep_mem, rep_mem_ps)
                    rep_r1_ps = psb.tile([128, 128], F32, tag="ps")
                    nc.tensor.matmul(rep_r1_ps, ones_row, r1row, start=True, stop=True)
                    rep_r1 = work.tile([128, 128], F32, tag="rep_r1")
                    nc.scalar.copy(rep_r1, rep_r1_ps)

                    d2 = work.tile([128, SEG], F32, tag="d2")
                    nc.vector.scalar_tensor_tensor(
                        out=d2, in0=dotp, scalar=ombc[:, h : h + 1], in1=rep_dot,
                        op0=ALU.mult, op1=ALU.mult,
                    )
                    m2 = work.tile([128, SEG], F32, tag="m2")
                    nc.vector.scalar_tensor_tensor(
                        out=m2, in0=memp, scalar=bgr1, in1=rep_mem,
                        op0=ALU.mult, op1=ALU.mult,
                    )
                    nc.vector.tensor_add(
                        af[:, :, h * 32 : (h + 1) * 32],
                        m2.rearrange("d (r t) -> d r t", t=32),
                        d2.rearrange("d (r t) -> d r t", t=32),
                    )

                    corr = psb.tile([128, 4 * DV], F32, tag="ps")
                    for c in range(4):
                        nc.tensor.matmul(
                            corr[:, c * DV : (c + 1) * DV],
                            skT[:, c * 128 : (c + 1) * 128],
                            mem[:, h, :],
                            start=True,
                            stop=True,
                        )
                    corrS = work.tile([128, 4, DV], F32R, tag="corrS")
                    for c in range(4):
                        nc.vector.tensor_scalar_mul(
                            corrS[:, c, :], corr[:, c * DV : (c + 1) * DV], rsk[:, c : c + 1]
                        )

                    sqc = work.tile([128, 4, 128], F32R, tag="sqc")
                    for c in range(4):
                        pt = psb.tile([128, 128], F32R, tag="ps")
                        nc.tensor.transpose(pt, sqT[:, c * 128 : (c + 1) * 128], ident)
                        if c % 2 == 0:
                            nc.scalar.copy(sqc[:, c, :], pt)
                        else:
                            nc.vector.tensor_copy(sqc[:, c, :], pt)

                    dAB = psb.tile([128, 2 * DV], F32, tag="ps")
                    for c in range(4):
                        nc.tensor.matmul(
                            dAB[:, 0:DV], sqc[:, c, :], vv[:, c, :],
                            start=(c == 0), stop=(c == 3),
                        )
                    for c in range(4):
                        nc.tensor.matmul(
                            dAB[:, DV : 2 * DV], sqc[:, c, :], corrS[:, c, :],
                            start=(c == 0), stop=(c == 3),
                        )
                    dB = work.tile([128, DV], F32, tag="dB")
                    nc.vector.tensor_mul(dB, dAB[:, DV : 2 * DV], rep_r1)
                    dA = work.tile([128, DV], F32, tag="dA")
                    nc.vector.tensor_sub(dA, dAB[:, 0:DV], dB)
                    nc.vector.tensor_add(mem[:, h, :], mem[:, h, :], dA)

                nc.sync.dma_start(out=af_scr[seg], in_=af.bitcast(F32))

        with ExitStack() as p3:
            wpool3 = p3.enter_context(tc.tile_pool(name="w_o", bufs=1))
            af_in = p3.enter_context(tc.tile_pool(name="af_in", bufs=2))
            outst = p3.enter_context(tc.tile_pool(name="outst3", bufs=3))
            ps3 = p3.enter_context(tc.tile_pool(name="ps3", bufs=6, space="PSUM"))

            wo_t = wpool3.tile([128, NKK, D], F32R, tag="wo")
            nc.sync.dma_start(
                out=wo_t, in_=wo.rearrange("(r p) m -> p r m", p=128).bitcast(F32R)
            )
            for seg in range(NSEG):
                r0 = seg * 256
                af3 = af_in.tile([128, NKK, 256], F32R, tag="af3")
                nc.sync.dma_start(out=af3, in_=af_scr[seg].bitcast(F32R))
                for st in range(2):
                    ost = outst.tile([128, D], F32, tag="ost")
                    for mc in range(4):
                        po = ps3.tile([128, 512], F32, tag="ps3")
                        for r in range(NKK):
                            nc.tensor.matmul(
                                po,
                                af3[:, r, st * 128 : (st + 1) * 128],
                                wo_t[:, r, mc * 512 : (mc + 1) * 512],
                                start=(r == 0),
                                stop=(r == NKK - 1),
                            )
                        if mc % 2 == 0:
                            nc.scalar.copy(ost[:, mc * 512 : (mc + 1) * 512], po)
                        else:
                            nc.vector.tensor_copy(ost[:, mc * 512 : (mc + 1) * 512], po)
                    nc.sync.dma_start(
                        out=out[r0 + st * 128 : r0 + (st + 1) * 128, :], in_=ost
                    )

    nc.finalize()
    return nc


_NC_CACHE = {}


def _get_nc():
    if "nc" not in _NC_CACHE:
        _NC_CACHE["nc"] = build_core_program()
    return _NC_CACHE["nc"]


def _make_in_maps(inputs):
    x = np.ascontiguousarray(np.asarray(inputs["x"], dtype=np.float32))
    Wq = np.ascontiguousarray(np.asarray(inputs["Wq"], dtype=np.float32))
    Wk = np.ascontiguousarray(np.asarray(inputs["Wk"], dtype=np.float32))
    Wv = np.ascontiguousarray(np.asarray(inputs["Wv"], dtype=np.float32))
    Wo = np.ascontiguousarray(np.asarray(inputs["Wo"], dtype=np.float32))
    betas = np.asarray(inputs["betas"], dtype=np.float32)

    bg = (1.0 / (1.0 + np.exp(-betas.astype(np.float64)))).astype(np.float32)
    bg = bg.reshape(16, 128)
    omb = (1.0 - bg).astype(np.float32)

    ident = np.eye(128, dtype=np.float32)
    ones = np.ones((128, 128), dtype=np.float32)
    zeros = np.zeros((128, NH * DV + NH), dtype=np.float32)

    in_maps = []
    for c in range(8):
        b, hh = c // 2, c % 2
        hsl = slice(hh * NH, (hh + 1) * NH)
        xp = np.ascontiguousarray(
            x[b].reshape(NSEG, 2, 256, D)[:, hh].reshape(TC, D)
        )
        in_maps.append(
            {
                "x": xp,
                "wq": Wq,
                "wk": Wk,
                "wv": Wv,
                "wo": Wo,
                "bg_cols": np.ascontiguousarray(bg[hsl].T),
                "omb_cols": np.ascontiguousarray(omb[hsl].T),
                "ident": ident,
                "ones_in": ones,
                "zeros_in": zeros,
            }
        )
    return in_maps


def kernel(x, Wq, Wk, Wv, Wo, betas):
    inputs = {"x": x, "Wq": Wq, "Wk": Wk, "Wv": Wv, "Wo": Wo, "betas": betas}
    in_maps = _make_in_maps(inputs)
    nc = _get_nc()
    res = run_bass_kernel_spmd(nc, in_maps, core_ids=list(range(8)))
    B = np.asarray(x).shape[0]
    out = np.empty((B, T, D), dtype=np.float32)
    for b in range(B):
        ob = out[b].reshape(NSEG, 2, 256, D)
        ob[:, 0] = res.results[2 * b]["out"].reshape(NSEG, 256, D)
        ob[:, 1] = res.results[2 * b + 1]["out"].reshape(NSEG, 256, D)
    return out

